# revision 10
# baseline (speedup 1.0000x reference)
"""StyleGAN2 up-2x blur (upfirdn2d, up=2, pad=(2,1), 4x4 kernel) on 8 trn2 cores.

x: (4, 64, 256, 256) f32, kernel: (4, 4) f32 -> out: (4, 64, 511, 511) f32.

Polyphase decomposition: out[2r+s, 2c+t] is a 2x2-tap conv of x with weights
from the flipped kernel w = kernel[::-1, ::-1]:
  s=0 -> vertical taps (w[0,kx] @ r-1, w[2,kx] @ r); s=1 -> (w[1,kx] @ r, w[3,kx] @ r+1)
  t=0 -> horizontal taps kx in {0 (c-1), 2 (c)};    t=1 -> kx in {1 (c), 3 (c+1)}

Sharding: pure data parallel over the 256 (N*C) planes, 32 planes/core.

The tolerance (2e-2) allows bf16 end-to-end: x is rounded to bf16 host-side
(the [1,3,3,1] filter weights are exactly bf16-representable; a w_lo
correction path exists for general kernels), matmuls accumulate in fp32
PSUM, and the output is written to HBM as bf16 (upconverted to f32 on the
host).  Measured rel err ~5e-3 vs the 2e-2 gate.  This halves both the load
and store HBM traffic vs the fp32/hi+lo baseline - which matters because the
kernel is pinned on the SDMA HBM-write path (~13.5 GB/s/engine, ~215 GB/s
aggregate, irrespective of packet size).

Device algorithm (per core): the vertical 2-tap combine runs on TensorE as
banded-matrix matmuls (stationary [128,128] bands) with the horizontal taps
folded in as column-shifted moving operands accumulating into the same PSUM
bank; two planes are packed per matmul (moving free = 512).  Output rows are
assembled as row-PAIRS per partition into an 8-plane "mega" SBUF tile
([127, 8, 2, 2, 511] bf16), so stores are 2 DMA ops of ~2 MB per 8 planes
(1016 descriptors of 2044 B each) - long per-engine packet runs that
amortize the per-op completion round-trip.  Rows 254/255/256 (the row-chunk
seam) are computed once for all 32 planes via diagonal matmuls over
plane-partitioned tiles and stored with one op.  Stores go through SWDGE
(nc.gpsimd) which spreads packets over all 16 SDMA engines; HWDGE stores
measured much slower.  Input is host-packed as [pair, r, g, c, w] so each
load op is one 256 KB DMA with 2 KB contiguous per-partition chunks.
"""

import os
import numpy as np
import ml_dtypes

_BF = ml_dtypes.bfloat16
_NCORES = 8
_PL = 32            # planes per core
_NPAIR = _PL // 2   # plane pairs per core
_MEGA = 4           # plane pairs per mega store tile (8 planes)
_H = 256
_W = 256
_OW = 511
_OP = 512     # padded output row pitch (32B-aligned DMA runs)

_cache = {}
last_exec_ns = None
last_results = None


def _build(wlo_nz: bool):
    from contextlib import ExitStack
    import concourse.mybir as mybir
    import concourse.tile as tile
    from concourse import bacc

    BF = mybir.dt.bfloat16
    F32 = mybir.dt.float32

    nc = bacc.Bacc("TRN2", target_bir_lowering=False, debug=False)
    # input planes, host-packed: [pair, r, g(plane in pair), c(row chunk), w]
    # with plane row = c*128 + r
    xl = nc.dram_tensor("xl", [_NPAIR, 128, 2, 2, _W], BF, kind="ExternalInput").ap()
    sth = nc.dram_tensor("sth", [128, 12, 128], BF, kind="ExternalInput").ap()
    dgh = nc.dram_tensor("dgh", [16, 16, 16], BF, kind="ExternalInput").ap()
    if wlo_nz:
        stl = nc.dram_tensor("stl", [128, 12, 128], BF, kind="ExternalInput").ap()
        dgl = nc.dram_tensor("dgl", [16, 16, 16], BF, kind="ExternalInput").ap()
    out = nc.dram_tensor("out", [_PL, _OW, _OP], BF, kind="ExternalOutput").ap()

    ncopy = 0  # alternate evacuation copies between VectorE and ScalarE
    nstore = 0  # round-robin store dispatch path

    with tile.TileContext(nc) as tc, ExitStack() as ctx:
        cpool = ctx.enter_context(tc.tile_pool(name="const", bufs=1))
        tpool = ctx.enter_context(tc.tile_pool(name="tin", bufs=4))
        epool = ctx.enter_context(tc.tile_pool(name="edge", bufs=1))
        mpool = ctx.enter_context(tc.tile_pool(name="mega", bufs=2))
        bpool = ctx.enter_context(tc.tile_pool(name="bnd", bufs=1))
        ppool = ctx.enter_context(tc.tile_pool(name="ps", bufs=8, space="PSUM"))

        sth_t = cpool.tile([128, 12, 128], BF)
        nc.sync.dma_start(out=sth_t[:, :, :], in_=sth)
        dgh_t = cpool.tile([16, 16, 16], BF)
        nc.sync.dma_start(out=dgh_t[:, :, :], in_=dgh)
        if wlo_nz:
            stl_t = cpool.tile([128, 12, 128], BF)
            nc.sync.dma_start(out=stl_t[:, :, :], in_=stl)
            dgl_t = cpool.tile([16, 16, 16], BF)
            nc.sync.dma_start(out=dgl_t[:, :, :], in_=dgl)

        def copy_out(dst, src):
            nonlocal ncopy
            if ncopy % 2 == 0:
                nc.vector.tensor_copy(out=dst, in_=src)
            else:
                nc.scalar.copy(out=dst, in_=src)
            ncopy += 1

        # ---- seam rows oy=254 (s0,r=127: x[126],x[127]), oy=255 (s1,r=127:
        # ---- x[127],x[128]), oy=256 (s0,r=128: x[127],x[128])
        # plane row R lives at xl[:, R % 128, :, R // 128, :]; partition = pair,
        # plane-in-pair g stays a free dim (16x16 diagonal stationaries per g)
        etiles = {}
        for row in (126, 127, 128):
            e = epool.tile([16, 2, _W], BF, tag=f"e{row}")
            nc.sync.dma_start(out=e[:, :, :],
                              in_=xl[:, row % 128, :, row // 128, :])
            etiles[row] = e

        bt = bpool.tile([16, 2, 3, _OP], BF)
        seams = ((0, ((0, 126), (2, 127))),   # oy254: ky=0 on x126, ky=2 on x127
                 (1, ((1, 127), (3, 128))),   # oy255
                 (2, ((0, 127), (2, 128))))   # oy256
        for g in (0, 1):
            for bi, taps in seams:
                pb = ppool.tile([16, 2, 256], F32, tag="ps")
                mms = []
                # (t_, kx, moving col slice, psum col slice)
                for t_, kx, mv, pc in ((0, 2, (0, 256), (0, 256)), (0, 0, (0, 255), (1, 256)),
                                       (1, 1, (0, 256), (0, 256)), (1, 3, (1, 256), (0, 255))):
                    for ky, erow in taps:
                        mms.append((t_, ky * 4 + kx, erow, mv, pc, "h"))
                        if wlo_nz:
                            mms.append((t_, ky * 4 + kx, erow, mv, pc, "l"))
                for i, (t_, j, erow, mv, pc, wp) in enumerate(mms):
                    dg = dgh_t if wp == "h" else dgl_t
                    nc.tensor.matmul(
                        pb[:, t_, pc[0]:pc[1]], dg[:, j, :],
                        etiles[erow][:, g, mv[0]:mv[1]],
                        start=(i == 0), stop=(i == len(mms) - 1))
                copy_out(bt[:, g, bi, 0:_OP:2], pb[:, 0, :])
                copy_out(bt[:, g, bi, 1:_OP:2], pb[:, 1, :])
        nc.gpsimd.dma_start(
            out=out[:, 254:257, :].rearrange("(p g) r w -> p g r w", g=2),
            in_=bt[:, :, :, :])

        # ---- main body: 4 megas x 4 pairs x 2 row-chunks
        # stationary groups: 0 = s0/chunkA (rows 0..126), 1 = s0/chunkB, 2 = s1
        for mega in range(_NPAIR // _MEGA):
            # mega assembly tile: [p, plane_in_mega, chunk, rowpair slot, col]
            m = mpool.tile([128, 2 * _MEGA, 2, 2, _OP], BF, tag="m")
            for pp in range(_MEGA):
                pair = mega * _MEGA + pp
                t = tpool.tile([128, 2, 2, _W], BF, tag="tin")
                nc.sync.dma_start(
                    out=t[:, :, :, :].rearrange("r g c w -> r (g c w)"),
                    in_=xl[pair].rearrange("r g c w -> r (g c w)"))
                for chunk in (0, 1):
                    ig0 = 0 if chunk == 0 else 1
                    # chunk A row-pair layout: [i,0]=s0A[i] (oy 2i),
                    # [i,1]=s1A[i] (oy 2i+1); chunk B: [i,0]=s1B[i]
                    # (oy 257+2i), [i,1]=s0B[i] (oy 258+2i)
                    rows = ((0, 1) if chunk == 0 else (1, 0))  # s feeding (slot0, slot1)
                    # psums: [s][t_] with 2 planes packed along the free dim
                    for s, ig in ((0, ig0), (1, 2)):
                        for t_, kxmv in ((0, ((2, (0, 256), (0, 256)), (0, (0, 255), (1, 256)))),
                                         (1, ((1, (0, 256), (0, 256)), (3, (1, 256), (0, 255))))):
                            pt = ppool.tile([128, 2, 256], F32, tag="ps")
                            mms = []
                            for kx, mv, pc in kxmv:
                                mms.append((ig * 4 + kx, mv, pc, "h"))
                                if wlo_nz:
                                    mms.append((ig * 4 + kx, mv, pc, "l"))
                            for i, (j, mv, pc, wp) in enumerate(mms):
                                st_ = sth_t if wp == "h" else stl_t
                                nc.tensor.matmul(
                                    pt[:, :, pc[0]:pc[1]], st_[:, j, :],
                                    t[:, :, chunk, mv[0]:mv[1]],
                                    start=(i == 0), stop=(i == len(mms) - 1))
                            # drain this psum group immediately (overlaps with
                            # the next group's matmuls), casting f32 -> bf16;
                            # both planes in one op (free dims [2, 256])
                            slot = rows.index(s)
                            dst = m[0:127, 2 * pp:2 * pp + 2, chunk, slot, :]
                            off = 0 if t_ == 0 else 1
                            copy_out(dst[:, :, off:_OP:2], pt[0:127, :, :])

            # stores: one op per (plane, chunk) with a simple 3-d AP
            # ([127, 2, 512], constant strides) - 4-d APs force SWDGE into
            # per-descriptor software emission and throttle the stream.
            # Ops round-robin over three independent dispatch paths
            # (HWDGE-ACT, SWDGE, HWDGE-SP): per-op dispatch + completion
            # serialize within one queue, so one queue caps at ~90 GB/s
            q0 = mega * 2 * _MEGA
            for pl in range(2 * _MEGA):
                for cb, (r0, r1) in enumerate(((0, 254), (257, 511))):
                    dst = out[q0 + pl, r0:r1, :] \
                        .rearrange("(p two) w -> p two w", two=2)
                    eng = (nc.scalar, nc.gpsimd, nc.sync)[nstore % 3]
                    eng.dma_start(out=dst, in_=m[0:127, pl, cb, :, :])
                    nstore += 1

    nc.compile()
    return nc


def _host_arrays(w):
    w = np.asarray(w, np.float32)
    w_hi = w.astype(_BF).astype(np.float32)
    w_lo = w - w_hi
    wlo_nz = bool(np.any(w_lo != 0))

    def build_st(wv):
        st = np.zeros((3, 4, 128, 128), np.float32)
        i6 = np.arange(126)
        i7 = np.arange(127)
        for kx in range(4):
            st[0, kx][i6, i6 + 1] = wv[0, kx]        # s0A subdiag, out rows 1..126
            st[0, kx][i7, i7] += wv[2, kx]           # s0A diag, out rows 0..126
            st[1, kx][i7, i7] = wv[0, kx]            # s0B diag
            st[1, kx][i7 + 1, i7] = wv[2, kx]        # s0B sub
            st[2, kx][i7, i7] = wv[1, kx]            # s1 diag
            st[2, kx][i7 + 1, i7] = wv[3, kx]        # s1 sub
        # [g,kx,p,i] -> [p, g*4+kx, i]
        return np.ascontiguousarray(
            st.reshape(12, 128, 128).transpose(1, 0, 2)).astype(_BF)

    def build_dg(wv):
        dg = np.zeros((4, 4, 16, 16), np.float32)
        i = np.arange(16)
        for ky in range(4):
            for kx in range(4):
                dg[ky, kx][i, i] = wv[ky, kx]
        return np.ascontiguousarray(
            dg.reshape(16, 16, 16).transpose(1, 0, 2)).astype(_BF)

    arrs = {"sth": build_st(w_hi), "dgh": build_dg(w_hi)}
    if wlo_nz:
        wlo_b = w_lo.astype(_BF).astype(np.float32)
        arrs["stl"] = build_st(wlo_b)
        arrs["dgl"] = build_dg(wlo_b)
    return wlo_nz, arrs


def kernel(x, kernel):
    global last_exec_ns, last_results
    from concourse.bass_utils import run_bass_kernel_spmd

    x = np.ascontiguousarray(np.asarray(x, np.float32))
    w = np.asarray(kernel, np.float32)[::-1, ::-1]
    wlo_nz, warrs = _host_arrays(w)

    if wlo_nz not in _cache:
        _cache[wlo_nz] = _build(wlo_nz)
    nc = _cache[wlo_nz]

    # pack planes -> [pair, r, g, c, w] bf16 (plane = 2*pair + g, row = c*128 + r)
    hi = x.reshape(_NCORES * _NPAIR, 2, 2, 128, _W).astype(_BF)
    xlk = np.ascontiguousarray(hi.transpose(0, 3, 1, 2, 4))

    in_maps = []
    for c in range(_NCORES):
        m = {"xl": xlk[c * _NPAIR:(c + 1) * _NPAIR]}
        m.update(warrs)
        in_maps.append(m)

    trace = bool(os.environ.get("BLUR_TRACE"))
    tmpdir = os.environ.get("BLUR_TRACE_DIR") or None
    if trace:
        try:
            res = run_bass_kernel_spmd(nc, in_maps, list(range(_NCORES)),
                                       trace=True, tmpdir=tmpdir)
            last_exec_ns = res.exec_time_ns
        except Exception as e:
            print(f"trace run failed ({type(e).__name__}: {e}); retrying untraced")
            res = run_bass_kernel_spmd(nc, in_maps, list(range(_NCORES)))
            last_exec_ns = None
    else:
        res = run_bass_kernel_spmd(nc, in_maps, list(range(_NCORES)))
        last_exec_ns = None
    last_results = res

    outs = np.stack([res.results[c]["out"] for c in range(_NCORES)])
    return outs[..., :_OW].reshape(4, 64, _OW, _OW).astype(np.float32)


# revision 12
# speedup vs baseline: 1.5880x; 1.5880x over previous
"""StyleGAN2 up-2x blur (upfirdn2d, up=2, pad=(2,1), 4x4 kernel) on 8 trn2 cores.

x: (4, 64, 256, 256) f32, kernel: (4, 4) f32 -> out: (4, 64, 511, 511) f32.

Polyphase decomposition: out[2r+s, 2c+t] is a 2x2-tap conv of x with weights
from the flipped kernel w = kernel[::-1, ::-1]:
  s=0 -> vertical taps (w[0,kx] @ r-1, w[2,kx] @ r); s=1 -> (w[1,kx] @ r, w[3,kx] @ r+1)
  t=0 -> horizontal taps kx in {0 (c-1), 2 (c)};    t=1 -> kx in {1 (c), 3 (c+1)}

Sharding: pure data parallel over the 256 (N*C) planes, 32 planes/core.

The tolerance (2e-2) allows bf16 end-to-end: x is rounded to bf16 host-side
(the [1,3,3,1] filter weights are exactly bf16-representable; a w_lo
correction path exists for general kernels), matmuls accumulate in fp32
PSUM, and the output is written to HBM as bf16 (upconverted to f32 on the
host).  Measured rel err ~5e-3 vs the 2e-2 gate.  This halves both the load
and store HBM traffic vs the fp32/hi+lo baseline - which matters because the
kernel is pinned on the SDMA HBM-write path (~13.5 GB/s/engine, ~215 GB/s
aggregate, irrespective of packet size).

Device algorithm (per core): the vertical 2-tap combine runs on TensorE as
banded-matrix matmuls (stationary [128,128] bands) with the horizontal taps
folded in as column-shifted moving operands accumulating into the same PSUM
bank; two planes are packed per matmul (moving free = 512).  Output rows are
assembled as row-PAIRS per partition into an 8-plane "mega" SBUF tile
([127, 8, 2, 2, 511] bf16), so stores are 2 DMA ops of ~2 MB per 8 planes
(1016 descriptors of 2044 B each) - long per-engine packet runs that
amortize the per-op completion round-trip.  Rows 254/255/256 (the row-chunk
seam) are computed once for all 32 planes via diagonal matmuls over
plane-partitioned tiles and stored with one op.  Stores go through SWDGE
(nc.gpsimd) which spreads packets over all 16 SDMA engines; HWDGE stores
measured much slower.  Input is host-packed as [pair, r, g, c, w] so each
load op is one 256 KB DMA with 2 KB contiguous per-partition chunks.
"""

import os
import numpy as np
import ml_dtypes

_BF = ml_dtypes.bfloat16
_NCORES = 8
_PL = 32            # planes per core
_NPAIR = _PL // 2   # plane pairs per core
_MEGA = 4           # plane pairs per mega store tile (8 planes)
_H = 256
_W = 256
_OW = 511
_OP = 512     # padded output row pitch (32B-aligned DMA runs)

_cache = {}
last_exec_ns = None
last_results = None


def _build(wlo_nz: bool):
    from contextlib import ExitStack
    import concourse.mybir as mybir
    import concourse.tile as tile
    from concourse import bacc

    BF = mybir.dt.bfloat16
    F32 = mybir.dt.float32

    nc = bacc.Bacc("TRN2", target_bir_lowering=False, debug=False)
    # input planes, host-packed: [pair, r, g(plane in pair), c(row chunk), w]
    # with plane row = c*128 + r
    xl = nc.dram_tensor("xl", [_NPAIR, 128, 2, 2, _W], BF, kind="ExternalInput").ap()
    sth = nc.dram_tensor("sth", [128, 12, 128], BF, kind="ExternalInput").ap()
    dgh = nc.dram_tensor("dgh", [16, 16, 16], BF, kind="ExternalInput").ap()
    if wlo_nz:
        stl = nc.dram_tensor("stl", [128, 12, 128], BF, kind="ExternalInput").ap()
        dgl = nc.dram_tensor("dgl", [16, 16, 16], BF, kind="ExternalInput").ap()
    out = nc.dram_tensor("out", [_PL, _OW, _OP], BF, kind="ExternalOutput").ap()

    ncopy = 0  # alternate evacuation copies between VectorE and ScalarE
    nstore = 0  # round-robin store dispatch path

    with tile.TileContext(nc) as tc, ExitStack() as ctx:
        cpool = ctx.enter_context(tc.tile_pool(name="const", bufs=1))
        tpool = ctx.enter_context(tc.tile_pool(name="tin", bufs=4))
        epool = ctx.enter_context(tc.tile_pool(name="edge", bufs=1))
        mpool = ctx.enter_context(tc.tile_pool(name="mega", bufs=2))
        bpool = ctx.enter_context(tc.tile_pool(name="bnd", bufs=1))
        ppool = ctx.enter_context(tc.tile_pool(name="ps", bufs=8, space="PSUM"))

        sth_t = cpool.tile([128, 12, 128], BF)
        nc.sync.dma_start(out=sth_t[:, :, :], in_=sth)
        dgh_t = cpool.tile([16, 16, 16], BF)
        nc.sync.dma_start(out=dgh_t[:, :, :], in_=dgh)
        if wlo_nz:
            stl_t = cpool.tile([128, 12, 128], BF)
            nc.sync.dma_start(out=stl_t[:, :, :], in_=stl)
            dgl_t = cpool.tile([16, 16, 16], BF)
            nc.sync.dma_start(out=dgl_t[:, :, :], in_=dgl)

        def copy_out(dst, src):
            nonlocal ncopy
            if ncopy % 2 == 0:
                nc.vector.tensor_copy(out=dst, in_=src)
            else:
                nc.scalar.copy(out=dst, in_=src)
            ncopy += 1

        # ---- seam rows oy=254 (s0,r=127: x[126],x[127]), oy=255 (s1,r=127:
        # ---- x[127],x[128]), oy=256 (s0,r=128: x[127],x[128])
        # plane row R lives at xl[:, R % 128, :, R // 128, :]; partition = pair,
        # plane-in-pair g stays a free dim (16x16 diagonal stationaries per g)
        etiles = {}
        for row in (126, 127, 128):
            e = epool.tile([16, 2, _W], BF, tag=f"e{row}")
            nc.sync.dma_start(out=e[:, :, :],
                              in_=xl[:, row % 128, :, row // 128, :])
            etiles[row] = e

        bt = bpool.tile([16, 2, 3, _OP], BF)
        seams = ((0, ((0, 126), (2, 127))),   # oy254: ky=0 on x126, ky=2 on x127
                 (1, ((1, 127), (3, 128))),   # oy255
                 (2, ((0, 127), (2, 128))))   # oy256
        for g in (0, 1):
            for bi, taps in seams:
                pb = ppool.tile([16, 2, 256], F32, tag="ps")
                mms = []
                # (t_, kx, moving col slice, psum col slice)
                for t_, kx, mv, pc in ((0, 2, (0, 256), (0, 256)), (0, 0, (0, 255), (1, 256)),
                                       (1, 1, (0, 256), (0, 256)), (1, 3, (1, 256), (0, 255))):
                    for ky, erow in taps:
                        mms.append((t_, ky * 4 + kx, erow, mv, pc, "h"))
                        if wlo_nz:
                            mms.append((t_, ky * 4 + kx, erow, mv, pc, "l"))
                for i, (t_, j, erow, mv, pc, wp) in enumerate(mms):
                    dg = dgh_t if wp == "h" else dgl_t
                    nc.tensor.matmul(
                        pb[:, t_, pc[0]:pc[1]], dg[:, j, :],
                        etiles[erow][:, g, mv[0]:mv[1]],
                        start=(i == 0), stop=(i == len(mms) - 1))
                copy_out(bt[:, g, bi, 0:_OP:2], pb[:, 0, :])
                copy_out(bt[:, g, bi, 1:_OP:2], pb[:, 1, :])
        nc.gpsimd.dma_start(
            out=out[:, 254:257, :].rearrange("(p g) r w -> p g r w", g=2),
            in_=bt[:, :, :, :])

        # ---- main body: 4 megas x 4 pairs x 2 row-chunks
        # stationary groups: 0 = s0/chunkA (rows 0..126), 1 = s0/chunkB, 2 = s1
        for mega in range(_NPAIR // _MEGA):
            # mega assembly tile: [p, plane_in_mega, chunk, rowpair slot, col]
            m = mpool.tile([128, 2 * _MEGA, 2, 2, _OP], BF, tag="m")
            for pp in range(_MEGA):
                pair = mega * _MEGA + pp
                t = tpool.tile([128, 2, 2, _W], BF, tag="tin")
                nc.sync.dma_start(
                    out=t[:, :, :, :].rearrange("r g c w -> r (g c w)"),
                    in_=xl[pair].rearrange("r g c w -> r (g c w)"))
                for chunk in (0, 1):
                    ig0 = 0 if chunk == 0 else 1
                    # chunk A row-pair layout: [i,0]=s0A[i] (oy 2i),
                    # [i,1]=s1A[i] (oy 2i+1); chunk B: [i,0]=s1B[i]
                    # (oy 257+2i), [i,1]=s0B[i] (oy 258+2i)
                    rows = ((0, 1) if chunk == 0 else (1, 0))  # s feeding (slot0, slot1)
                    # psums: [s][t_] with 2 planes packed along the free dim
                    for s, ig in ((0, ig0), (1, 2)):
                        for t_, kxmv in ((0, ((2, (0, 256), (0, 256)), (0, (0, 255), (1, 256)))),
                                         (1, ((1, (0, 256), (0, 256)), (3, (1, 256), (0, 255))))):
                            pt = ppool.tile([128, 2, 256], F32, tag="ps")
                            mms = []
                            for kx, mv, pc in kxmv:
                                mms.append((ig * 4 + kx, mv, pc, "h"))
                                if wlo_nz:
                                    mms.append((ig * 4 + kx, mv, pc, "l"))
                            for i, (j, mv, pc, wp) in enumerate(mms):
                                st_ = sth_t if wp == "h" else stl_t
                                nc.tensor.matmul(
                                    pt[:, :, pc[0]:pc[1]], st_[:, j, :],
                                    t[:, :, chunk, mv[0]:mv[1]],
                                    start=(i == 0), stop=(i == len(mms) - 1))
                            # drain this psum group immediately (overlaps with
                            # the next group's matmuls), casting f32 -> bf16;
                            # both planes in one op (free dims [2, 256])
                            slot = rows.index(s)
                            dst = m[0:127, 2 * pp:2 * pp + 2, chunk, slot, :]
                            off = 0 if t_ == 0 else 1
                            copy_out(dst[:, :, off:_OP:2], pt[0:127, :, :])

            # stores: one SWDGE op per (pair, chunk): dst [127, 2 planes,
            # 1024 elems] - 3-d AP with 2048B aligned runs.  HWDGE stores
            # dispatch at ~10.5us/op serialized on the issuing engine (dead
            # end), and SWDGE ops pace at ~2.8us each regardless of size,
            # so fewer/bigger ops win; 4-d APs hit a ~10x slower Q7
            # descriptor-emission path, so keep APs 3-d
            q0 = mega * 2 * _MEGA
            for pp in range(_MEGA):
                for cb, (r0, r1) in enumerate(((0, 254), (257, 511))):
                    dst = out[q0 + 2 * pp:q0 + 2 * pp + 2, r0:r1, :] \
                        .rearrange("g (p two) w -> p g (two w)", two=2)
                    src_ = m[0:127, 2 * pp:2 * pp + 2, cb, :, :] \
                        .rearrange("p g two w -> p g (two w)")
                    nc.gpsimd.dma_start(out=dst, in_=src_)
                    nstore += 1

    nc.compile()
    return nc


def _host_arrays(w):
    w = np.asarray(w, np.float32)
    w_hi = w.astype(_BF).astype(np.float32)
    w_lo = w - w_hi
    wlo_nz = bool(np.any(w_lo != 0))

    def build_st(wv):
        st = np.zeros((3, 4, 128, 128), np.float32)
        i6 = np.arange(126)
        i7 = np.arange(127)
        for kx in range(4):
            st[0, kx][i6, i6 + 1] = wv[0, kx]        # s0A subdiag, out rows 1..126
            st[0, kx][i7, i7] += wv[2, kx]           # s0A diag, out rows 0..126
            st[1, kx][i7, i7] = wv[0, kx]            # s0B diag
            st[1, kx][i7 + 1, i7] = wv[2, kx]        # s0B sub
            st[2, kx][i7, i7] = wv[1, kx]            # s1 diag
            st[2, kx][i7 + 1, i7] = wv[3, kx]        # s1 sub
        # [g,kx,p,i] -> [p, g*4+kx, i]
        return np.ascontiguousarray(
            st.reshape(12, 128, 128).transpose(1, 0, 2)).astype(_BF)

    def build_dg(wv):
        dg = np.zeros((4, 4, 16, 16), np.float32)
        i = np.arange(16)
        for ky in range(4):
            for kx in range(4):
                dg[ky, kx][i, i] = wv[ky, kx]
        return np.ascontiguousarray(
            dg.reshape(16, 16, 16).transpose(1, 0, 2)).astype(_BF)

    arrs = {"sth": build_st(w_hi), "dgh": build_dg(w_hi)}
    if wlo_nz:
        wlo_b = w_lo.astype(_BF).astype(np.float32)
        arrs["stl"] = build_st(wlo_b)
        arrs["dgl"] = build_dg(wlo_b)
    return wlo_nz, arrs


def kernel(x, kernel):
    global last_exec_ns, last_results
    from concourse.bass_utils import run_bass_kernel_spmd

    x = np.ascontiguousarray(np.asarray(x, np.float32))
    w = np.asarray(kernel, np.float32)[::-1, ::-1]
    wlo_nz, warrs = _host_arrays(w)

    if wlo_nz not in _cache:
        _cache[wlo_nz] = _build(wlo_nz)
    nc = _cache[wlo_nz]

    # pack planes -> [pair, r, g, c, w] bf16 (plane = 2*pair + g, row = c*128 + r)
    hi = x.reshape(_NCORES * _NPAIR, 2, 2, 128, _W).astype(_BF)
    xlk = np.ascontiguousarray(hi.transpose(0, 3, 1, 2, 4))

    in_maps = []
    for c in range(_NCORES):
        m = {"xl": xlk[c * _NPAIR:(c + 1) * _NPAIR]}
        m.update(warrs)
        in_maps.append(m)

    trace = bool(os.environ.get("BLUR_TRACE"))
    tmpdir = os.environ.get("BLUR_TRACE_DIR") or None
    if trace:
        try:
            res = run_bass_kernel_spmd(nc, in_maps, list(range(_NCORES)),
                                       trace=True, tmpdir=tmpdir)
            last_exec_ns = res.exec_time_ns
        except Exception as e:
            print(f"trace run failed ({type(e).__name__}: {e}); retrying untraced")
            res = run_bass_kernel_spmd(nc, in_maps, list(range(_NCORES)))
            last_exec_ns = None
    else:
        res = run_bass_kernel_spmd(nc, in_maps, list(range(_NCORES)))
        last_exec_ns = None
    last_results = res

    outs = np.stack([res.results[c]["out"] for c in range(_NCORES)])
    return outs[..., :_OW].reshape(4, 64, _OW, _OW).astype(np.float32)
